# revision 4
# baseline (speedup 1.0000x reference)
"""Contrastive loss (InfoNCE-style logsumexp of cosine-similarity matrix) on
8 Trainium2 NeuronCores.

loss = -mean_i logsumexp_j( cos(z1_i, z2_j) / 0.05 ),  z1,z2: [8192, 512] f32

Strategy: shard z1 row-wise (1024 rows/core), replicate z2. Each core:
  1. loads its z1 shard + full z2 (1 MiB batched DMAs), row sum-of-squares
     via DVE tensor_tensor_reduce, Sqrt (ACT, batched) + DVE reciprocal,
     row-scales on GpSimd (idle engine; folds 1/0.05 into z1),
  2. PE-transposes to d-major layout into 4-bank PSUM tiles; one batched
     PSUM->SBUF CAST converts to float32r (rounded fp32, 11-bit mantissa ->
     4x faster PE datapath),
  3. sim = z1h @ z2h.T as f32r matmuls (K=512 via 4 accumulating chunks),
     4 j-groups of 512 into one [128, 2048] 4-bank PSUM tile,
  4. one ACT Exp per 4-bank tile, in place, with fused row-sum (accum_out);
     logsumexp without max-subtraction (|sim| <= 20 -> exp <= 5e8, safe),
  5. reduce + Ln -> per-row lse [128, 8] -> DRAM.
Host gathers the 8 lse tiles and returns -mean.
"""
import sys

sys.path.insert(0, "/opt/trn_rl_repo")
import numpy as np
import concourse.bacc as bacc
import concourse.mybir as mybir
from concourse import tile, masks
from concourse.bass_utils import run_bass_kernel_spmd

F32 = mybir.dt.float32
F32R = mybir.dt.float32r
AF = mybir.ActivationFunctionType
ALU = mybir.AluOpType

N, D, C = 8192, 512, 8
NS = N // C            # 1024 z1 rows per core
IB = NS // 128         # 8 i-blocks per core
NB2 = N // 128         # 64 z2 row-blocks
JG = 16                # j-groups of 512 columns
JT = 4                 # j-tiles of 2048 columns (4 PSUM banks)
INV_TEMP = 20.0        # 1 / 0.05


def _build():
    nc = bacc.Bacc("TRN2", target_bir_lowering=False, debug=False, num_devices=C)
    z1_d = nc.dram_tensor("z1s", [NS, D], F32, kind="ExternalInput").ap()
    z2_d = nc.dram_tensor("z2", [N, D], F32, kind="ExternalInput").ap()
    lse_d = nc.dram_tensor("lse", [128, IB], F32, kind="ExternalOutput").ap()

    with tile.TileContext(nc) as tc:
        with (
            tc.tile_pool(name="const", bufs=1) as cpool,
            tc.tile_pool(name="stage", bufs=3) as stg,
            tc.tile_pool(name="hat", bufs=4) as hat,
            tc.tile_pool(name="sqs", bufs=2) as sqs,
            tc.tile_pool(name="pbig", bufs=2, space="PSUM") as pbig,
        ):
            ident = cpool.tile([128, 128], F32)
            masks.make_identity(nc, ident[:])

            z1T = cpool.tile([128, 4 * NS], F32R, name="z1T")    # [d, (k, i)]
            z2T = cpool.tile([128, 4 * N], F32R, name="z2T")     # [d, (k, j)]
            # (k, i) block views for matmul operands
            z1Tk = z1T[:].rearrange("p (k i) -> p k i", k=4)
            z2Tk = z2T[:].rearrange("p (k j) -> p k j", k=4)
            # (block, k, i) views for the batched transpose copies
            z1Tb = z1T[:].rearrange("p (k nb i) -> p nb k i", k=4, i=128)
            z2Tb = z2T[:].rearrange("p (k nb i) -> p nb k i", k=4, i=128)

            n1sq = cpool.tile([128, IB], F32, name="n1sq")
            n1s = cpool.tile([128, IB], F32, name="n1s")
            rn1 = cpool.tile([128, IB], F32, name="rn1")
            n2sq = cpool.tile([128, NB2], F32, name="n2sq")
            n2s = cpool.tile([128, NB2], F32, name="n2s")
            rn2 = cpool.tile([128, NB2], F32, name="rn2")
            esums = cpool.tile([128, IB * JT], F32, name="esums")
            stot = cpool.tile([128, IB], F32, name="stot")
            lse_s = cpool.tile([128, IB], F32, name="lse_s")

            def sumsq(st, n, nsq_col, b):
                # row sum-of-squares of one [128, 512] block; split across
                # engines (ACT fused square+accum vs GpSimd mult + DVE reduce)
                blk = st[:, n * D:(n + 1) * D]
                if b % 2 == 0:
                    sq = sqs.tile([128, D], F32, tag="sq", name="sq_scr")
                    nc.scalar.activation(sq[:], blk, AF.Square, accum_out=nsq_col)
                else:
                    sq = sqs.tile([128, D], F32, tag="sq", name="sq_scr")
                    nc.gpsimd.tensor_mul(sq[:], blk, blk)
                    nc.vector.reduce_sum(nsq_col, sq[:], axis=mybir.AxisListType.X)

            def prep4(st, rn, b0, dstv, scale2):
                # normalize 4 blocks (GpSimd), transpose (PE), 1 CAST out
                ps = pbig.tile([128, 2048], F32, tag="big", name="ps_t")
                for n in range(4):
                    b = b0 + n
                    zh = hat.tile([128, D], F32, tag="hat", name="zh")
                    if scale2 is None:
                        nc.gpsimd.tensor_scalar_mul(
                            zh[:], st[:, n * D:(n + 1) * D], rn[:, b:b + 1])
                    else:
                        nc.gpsimd.tensor_scalar(
                            zh[:], st[:, n * D:(n + 1) * D],
                            rn[:, b:b + 1], scale2, op0=ALU.mult, op1=ALU.mult)
                    for k in range(4):
                        nc.tensor.transpose(
                            ps[:, (n * 4 + k) * 128:(n * 4 + k + 1) * 128],
                            zh[:, k * 128:(k + 1) * 128], ident[:])
                return ps

            psv = "p (nb k i) -> p nb k i"

            # ---------- z1 shard: 2 groups of 4 blocks
            z1r = z1_d.rearrange("(g n p) d -> g p n d", n=4, p=128)
            z1st = []
            for g in range(2):
                st = stg.tile([128, 4 * D], F32, tag="stage", name=f"st1_{g}")
                nc.sync.dma_start(out=st[:].rearrange("p (n d) -> p n d", n=4),
                                  in_=z1r[g])
                z1st.append(st)
                for n in range(4):
                    sumsq(st, n, n1sq[:, 4 * g + n:4 * g + n + 1], 4 * g + n)
            nc.scalar.activation(n1s[:], n1sq[:], AF.Sqrt)
            nc.vector.reciprocal(rn1[:], n1s[:])
            for g in range(2):
                ps = prep4(z1st[g], rn1, 4 * g, z1Tb, INV_TEMP)
                nc.scalar.copy(z1Tb[:, 4 * g:4 * g + 4],
                               ps[:].rearrange(psv, nb=4, k=4))

            # ---------- z2 full: 16 groups of 4 blocks; sqrt batched per 2 groups
            z2r = z2_d.rearrange("(g n p) d -> g p n d", n=4, p=128)
            z2st = {}
            for g in range(JG):
                st = stg.tile([128, 4 * D], F32, tag="stage", name=f"st2_{g}")
                nc.sync.dma_start(out=st[:].rearrange("p (n d) -> p n d", n=4),
                                  in_=z2r[g])
                z2st[g] = st
                for n in range(4):
                    sumsq(st, n, n2sq[:, 4 * g + n:4 * g + n + 1], 4 * g + n)
                if g % 2 == 1:
                    s8 = slice(4 * (g - 1), 4 * (g + 1))
                    nc.scalar.activation(n2s[:, s8], n2sq[:, s8], AF.Sqrt)
                    nc.vector.reciprocal(rn2[:, s8], n2s[:, s8])
                    for gg in (g - 1, g):
                        ps = prep4(z2st.pop(gg), rn2, 4 * gg, z2Tb, None)
                        nc.vector.tensor_copy(z2Tb[:, 4 * gg:4 * gg + 4],
                                              ps[:].rearrange(psv, nb=4, k=4))

            # ---------- main: sim tiles of [128, 2048] + one Exp each
            for ib in range(IB):
                for jt in range(JT):
                    ps = pbig.tile([128, 2048], F32, tag="big", name=f"mm{ib}_{jt}")
                    for k in range(4):
                        for jq in range(4):
                            jb = jt * 4 + jq
                            nc.tensor.matmul(
                                ps[:, jq * 512:(jq + 1) * 512],
                                lhsT=z1Tk[:, k, ib * 128:(ib + 1) * 128],
                                rhs=z2Tk[:, k, jb * 512:(jb + 1) * 512],
                                start=(k == 0), stop=(k == 3),
                                skip_group_check=True)
                    nc.scalar.activation(
                        ps[:], ps[:], AF.Exp,
                        accum_out=esums[:, ib * JT + jt:ib * JT + jt + 1])

            # ---------- logsumexp tail
            nc.vector.reduce_sum(stot[:], esums[:].rearrange("p (a b) -> p a b", b=JT),
                                 axis=mybir.AxisListType.X)
            nc.scalar.activation(lse_s[:], stot[:], AF.Ln)
            nc.sync.dma_start(out=lse_d[:], in_=lse_s[:])

    nc.compile()
    return nc


_nc = None


def _get_nc():
    global _nc
    if _nc is None:
        _nc = _build()
    return _nc


def kernel(z1: np.ndarray, z2: np.ndarray, _trace: bool = False, **_):
    nc = _get_nc()
    z1 = np.ascontiguousarray(z1, dtype=np.float32)
    z2 = np.ascontiguousarray(z2, dtype=np.float32)
    in_maps = [
        {"z1s": z1[c * NS:(c + 1) * NS], "z2": z2} for c in range(C)
    ]
    res = run_bass_kernel_spmd(nc, in_maps, list(range(C)), trace=_trace)
    total = 0.0
    for c in range(C):
        total += res.results[c]["lse"].astype(np.float64).sum()
    out = np.float32(-(total / N))
    if _trace:
        return out, res
    return out
